# revision 1
# baseline (speedup 1.0000x reference)
"""LocalIsing energy kernel for Trainium2 (8 NeuronCores, data-parallel over batch).

reference:  energy[b] = x[b] @ J1 + sum_c J2[c] * x[b, p0[c]] * x[b, p1[c]]

The pair term is a quadratic form: scatter-add J2 into W[512,512] at (p0,p1)
(host-side, cheap: 130816 elements), then
    energy[b] = sum_j x[b,j] * (x @ W)[b,j]  +  sum_j x[b,j] * J1[j]
Each core handles 128 batch rows: a [128,512] @ [512,512] fp32 matmul on the
PE (4 accumulating K-tiles) plus two fused multiply+reduce DVE instructions.

Input packing (host side) keeps every device instruction to <=1 sync wait
(PE Matmult supports only one):
  wxt [4,128,640]: per K-tile, W rows (512) || x^T rows (128)  -> one DMA
  xj  [128,1024]:  x shard (512) || J1 broadcast (512)         -> one DMA
"""

import numpy as np
from contextlib import ExitStack

import concourse.tile as tile
from concourse import bacc, mybir
from concourse.bass_utils import run_bass_kernel_spmd

N = 512          # spins
B = 1024         # batch
NCORES = 8
BS = B // NCORES  # 128 rows per core = one partition tile
KT = N // 128     # 4 contraction tiles

_cached_nc = None


def _build():
    f32 = mybir.dt.float32
    nc = bacc.Bacc(
        "TRN2", target_bir_lowering=False, debug=False, num_devices=1
    )
    wxt = nc.dram_tensor("wxt", [KT, 128, N + BS], f32, kind="ExternalInput")
    xj = nc.dram_tensor("xj", [BS, 2 * N], f32, kind="ExternalInput")
    en = nc.dram_tensor("energy", [BS, 1], f32, kind="ExternalOutput")

    wxt_r = wxt.rearrange("k p n -> p k n")

    with tile.TileContext(nc) as tc, ExitStack() as ctx:
        sb = ctx.enter_context(tc.tile_pool(name="sb", bufs=1))
        ps = ctx.enter_context(tc.tile_pool(name="ps", bufs=1, space="PSUM"))

        wxt_sb = sb.tile([128, KT, N + BS], f32)
        nc.sync.dma_start(wxt_sb[:, :, :], wxt_r[:, :, :])
        xj_sb = sb.tile([128, 2 * N], f32)
        nc.sync.dma_start(xj_sb, xj[:, :])

        # e1[b] = sum_j x[b,j] * J1[j]
        scr1 = sb.tile([128, N], f32)
        e1 = sb.tile([128, 1], f32)
        nc.vector.tensor_mul(scr1, xj_sb[:, :N], xj_sb[:, N:])
        nc.vector.tensor_reduce(
            e1, scr1, axis=mybir.AxisListType.X, op=mybir.AluOpType.add
        )

        # y = x @ W   (4 accumulating K-tiles on the PE)
        y = ps.tile([128, N], f32)
        for k in range(KT):
            nc.tensor.matmul(
                y,
                wxt_sb[:, k, N:],      # lhsT = x^T K-tile [128, 128]
                wxt_sb[:, k, :N],      # rhs  = W  K-tile [128, 512]
                start=(k == 0),
                stop=(k == KT - 1),
            )

        # e2[b] = sum_j y[b,j] * x[b,j] ; e = e1 + e2
        scr2 = sb.tile([128, N], f32)
        e2 = sb.tile([128, 1], f32)
        nc.vector.tensor_mul(scr2, y, xj_sb[:, :N])
        nc.vector.tensor_reduce(
            e2, scr2, axis=mybir.AxisListType.X, op=mybir.AluOpType.add
        )
        e = sb.tile([128, 1], f32)
        nc.vector.tensor_add(e, e1, e2)
        nc.sync.dma_start(en[:, :], e)
    nc.finalize()
    return nc


def _pack_inputs(x, J1, J2, pairs):
    x = np.asarray(x, dtype=np.float32)
    J1 = np.asarray(J1, dtype=np.float32)
    J2f = np.asarray(J2, dtype=np.float64)
    pairs = np.asarray(pairs)

    # Scatter-add J2 into W (handles duplicate pairs exactly like the
    # reference's gather-sum).
    idx = pairs[:, 0].astype(np.int64) * N + pairs[:, 1].astype(np.int64)
    W = np.bincount(idx, weights=J2f, minlength=N * N).astype(np.float32)
    W = W.reshape(KT, 128, N)

    in_maps = []
    for c in range(NCORES):
        shard = x[c * BS : (c + 1) * BS]
        wxt = np.concatenate([W, shard.T.reshape(KT, 128, BS)], axis=2)
        xj = np.concatenate([shard, np.broadcast_to(J1, (BS, N))], axis=1)
        in_maps.append(
            {"wxt": np.ascontiguousarray(wxt), "xj": np.ascontiguousarray(xj)}
        )
    return in_maps


def kernel(x, J1, J2, pairs):
    global _cached_nc
    if _cached_nc is None:
        _cached_nc = _build()
    in_maps = _pack_inputs(x, J1, J2, pairs)
    res = run_bass_kernel_spmd(_cached_nc, in_maps, core_ids=list(range(NCORES)))
    return np.concatenate([r["energy"].reshape(-1) for r in res.results])



# revision 3
# speedup vs baseline: 1.7103x; 1.7103x over previous
"""LocalIsing energy kernel for Trainium2 (8 NeuronCores, data-parallel over batch).

reference:  energy[b] = x[b] @ J1 + sum_c J2[c] * x[b, p0[c]] * x[b, p1[c]]

The pair term is a quadratic form: scatter-add J2 into W[512,512] at (p0,p1)
(host-side, cheap), then
    energy[b] = sum_j x[b,j] * ((x @ W)[b,j] + J1[j])
J1 rides along as a K=1 matmul tile (ones row x J1 row) accumulated into the
same PSUM bank, so the whole energy is one fused DVE multiply+reduce.

All operands travel as bf16 (x is exactly representable; W/J1 rounding gives
~0.3% relative error, far under the 2e-2 gate). Per core one packed DRAM blob
[128, 3200] bf16 keeps every partition line contiguous (6400B descriptors):
  per partition p: W rows (4 K-tiles x 512) | x^T cols (4 x 128) | x row (512)
                   | identity row (128, for the output transpose)
The [128,1] energy column is PE-transposed to [1,128] so the result leaves in
a single 256B DMA packet instead of 128 4-byte packets.
"""

import numpy as np
from contextlib import ExitStack

import ml_dtypes
import concourse.tile as tile
from concourse import bacc, mybir
from concourse.bass_utils import run_bass_kernel_spmd

N = 512          # spins
B = 1024         # batch
NCORES = 8
BS = B // NCORES  # 128 rows per core = one partition tile
KT = N // 128     # 4 contraction tiles

BF16 = ml_dtypes.bfloat16

# blob column offsets (bf16 elements)
_W_OFF = 0                 # 4 tiles x 512
_XT_OFF = KT * N           # 2048: 4 tiles x 128
_X_OFF = _XT_OFF + N       # 2560: x row (512)
_ID_OFF = _X_OFF + N       # 3072: identity row (128)
_BLOB_W = _ID_OFF + 128    # 3200

_cached_nc = None


def _build():
    bf16 = mybir.dt.bfloat16
    f32 = mybir.dt.float32
    nc = bacc.Bacc(
        "TRN2", target_bir_lowering=False, debug=False, num_devices=1
    )
    blob = nc.dram_tensor("blob", [128, _BLOB_W], bf16, kind="ExternalInput")
    cst = nc.dram_tensor("cst", [1, N + 128], bf16, kind="ExternalInput")
    en = nc.dram_tensor("energy", [1, BS], bf16, kind="ExternalOutput")

    with tile.TileContext(nc) as tc, ExitStack() as ctx:
        sb = ctx.enter_context(tc.tile_pool(name="sb", bufs=1))
        ps = ctx.enter_context(tc.tile_pool(name="ps", bufs=1, space="PSUM"))

        cst_sb = sb.tile([1, N + 128], bf16)
        nc.sync.dma_start(cst_sb, cst[:, :])
        blob_sb = sb.tile([128, _BLOB_W], bf16)
        nc.sync.dma_start(blob_sb, blob[:, :])

        # y = 1 (x) J1  +  x @ W   (K=1 tile first: only needs cst, so it can
        # issue while the blob DMA is still in flight)
        y = ps.tile([128, N], f32)
        nc.tensor.matmul(
            y, cst_sb[:1, N : N + 128], cst_sb[:1, :N], start=True, stop=False
        )
        for k in range(KT):
            nc.tensor.matmul(
                y,
                blob_sb[:, _XT_OFF + k * 128 : _XT_OFF + (k + 1) * 128],
                blob_sb[:, _W_OFF + k * N : _W_OFF + (k + 1) * N],
                start=False,
                stop=(k == KT - 1),
            )

        # e[b] = sum_j y[b,j] * x[b,j]  (single fused DVE mul+reduce;
        # tensor_tensor_reduce miscompiles on HW, scalar_tensor_tensor's
        # accum_out path does not)
        scr = sb.tile([128, N], f32)
        e32 = sb.tile([128, 1], f32)
        nc.vector.scalar_tensor_tensor(
            out=scr,
            in0=y,
            scalar=1.0,
            in1=blob_sb[:, _X_OFF : _X_OFF + N],
            op0=mybir.AluOpType.mult,
            op1=mybir.AluOpType.mult,
            accum_out=e32,
        )

        # [128,1] -> [1,128] via PE transpose so the output leaves as one packet
        e16 = sb.tile([128, 1], bf16)
        nc.vector.tensor_copy(e16, e32)
        et = ps.tile([1, 128], bf16)
        nc.tensor.transpose(et, e16, blob_sb[:, _ID_OFF : _ID_OFF + 128])
        erow = sb.tile([1, 128], bf16)
        nc.vector.tensor_copy(erow, et)
        nc.sync.dma_start(en[:, :], erow)
    nc.finalize()
    return nc


def _pack_inputs(x, J1, J2, pairs):
    x = np.asarray(x, dtype=np.float32)
    J1 = np.asarray(J1, dtype=np.float32)
    J2f = np.asarray(J2, dtype=np.float64)
    pairs = np.asarray(pairs)

    # Scatter-add J2 into W (handles duplicate/diagonal pairs exactly like the
    # reference's gather-sum).
    idx = pairs[:, 0].astype(np.int64) * N + pairs[:, 1].astype(np.int64)
    W = np.bincount(idx, weights=J2f, minlength=N * N).astype(np.float32)
    Wb = W.reshape(N, N).astype(BF16)
    # [KT,128,512] -> partition-major [128, KT*512]
    Wrows = np.ascontiguousarray(
        Wb.reshape(KT, 128, N).transpose(1, 0, 2).reshape(128, KT * N)
    )
    eye = np.eye(128, dtype=BF16)
    cst = np.concatenate([J1.astype(BF16), np.ones(128, dtype=BF16)])[None, :]

    in_maps = []
    for c in range(NCORES):
        shard = x[c * BS : (c + 1) * BS].astype(BF16)
        blob = np.empty((128, _BLOB_W), dtype=BF16)
        blob[:, _W_OFF:_XT_OFF] = Wrows
        # lhsT tile k, partition p holds x_shard[:, 128k+p]
        blob[:, _XT_OFF:_X_OFF] = np.ascontiguousarray(
            shard.T.reshape(KT, 128, BS).transpose(1, 0, 2).reshape(128, KT * BS)
        )
        blob[:, _X_OFF:_ID_OFF] = shard
        blob[:, _ID_OFF:] = eye
        in_maps.append({"blob": blob, "cst": cst})
    return in_maps


def kernel(x, J1, J2, pairs):
    global _cached_nc
    if _cached_nc is None:
        _cached_nc = _build()
    in_maps = _pack_inputs(x, J1, J2, pairs)
    res = run_bass_kernel_spmd(_cached_nc, in_maps, core_ids=list(range(NCORES)))
    return np.concatenate(
        [r["energy"].reshape(-1).astype(np.float32) for r in res.results]
    )
